# revision 7
# baseline (speedup 1.0000x reference)
"""GAT message-passing kernel for 8 trn2 NeuronCores.

Math (reference):
    Wx = x @ W;  s1 = Wx@a1/sqrt(2D);  s2 = Wx@a2/sqrt(2D)   (per t)
    weight = softmax_m(lrelu(s1[m] + s2[n]));  agg = lrelu(weight @ Wx)
    out = x - agg

Key identities:
  * exp(lrelu(s1+s2)) rescaled by exp(-s2) gives per-(m,n) weight
        et[m,n] = max(E1[m], F1[m] * r[n])
    with E1=exp(s1), F1=exp(0.01 s1), r=exp(-0.99 s2) - O(N) exps only.
  * et = E1[m] + relu(F1[m]*r[n] - E1[m]), so the score tile can be
    produced EITHER as one DVE/Pool tensor_scalar (mult,max) or as one
    ACT Relu with per-partition scale/bias; the E1 part for ACT-form
    tiles is a rank-1 correction folded into the PSUM accumulation via
    a 1-partition broadcast matmul.
  * softmax denominator = ones column appended to Wx (moving operand).
  * out = x - prelu(agg, 0.01).

Sharding: 8 cores = 4 t-slices x 2 N-halves; each core aggregates over
all 4096 source nodes for its own (t, 2048 dest nodes). Whole data
path in bf16 (inputs packed bf16 host-side, output bf16 -> f32).
"""

import sys

if "/opt/trn_rl_repo" not in sys.path:
    sys.path.insert(0, "/opt/trn_rl_repo")

import numpy as np

N, T, D = 4096, 4, 128
P = 128
HALF = N // 2            # 2048 dest nodes per core
MT = N // P              # 32 m tiles
NT = HALF // P           # 16 own n tiles
SCALE_INV = 1.0 / 16.0   # 1/sqrt(2*128)

# packed bf16 input column offsets: [W | WT | av | xT | xn]
C_W = 0
C_WT = D                 # 128
C_AV = 2 * D             # 256
C_XT = 2 * D + 2         # 258
C_XN = C_XT + N          # 4354
C_END = C_XN + HALF      # 6402

# per-mt engine for the [P, 1024] score tile: v=DVE, a=ACT(relu form),
# p=Pool/gpsimd. ACT-form tiles need the rank-1 E1 correction.
MT_ENG = list("vvavvavp" * 3 + "vvavavap")
ACT_MTS = [i for i, e in enumerate(MT_ENG) if e == "a"]

_CACHE = {}


def _build():
    import concourse.mybir as mybir
    from concourse import bacc
    from concourse.tile import TileContext

    f32 = mybir.dt.float32
    bf16 = mybir.dt.bfloat16
    Alu = mybir.AluOpType
    Act = mybir.ActivationFunctionType

    nc = bacc.Bacc()
    xin = nc.declare_dram_parameter("xin", [P, C_END], bf16, isOutput=False)
    out = nc.declare_dram_parameter("out", [HALF, D], bf16, isOutput=True)

    with TileContext(nc) as tc:
        with (
            tc.tile_pool(name="const", bufs=1) as cpool,
            tc.tile_pool(name="epool", bufs=6) as epool,
            tc.tile_pool(name="fpool", bufs=4) as fpool,
            tc.tile_pool(name="opool", bufs=2) as opool,
        ):
            # ---- input DMAs ----
            px0 = cpool.tile([P, C_XT + 1024], bf16)
            nc.sync.dma_start(px0[:, :], xin[:, 0 : C_XT + 1024])
            prm = px0[:, 0:C_XT]
            xts = [px0[:, C_XT : C_XT + 1024]]
            for ch in range(1, 4):
                xt_c = cpool.tile([P, 1024], bf16, name=f"xt{ch}", tag=f"xt{ch}")
                nc.sync.dma_start(
                    xt_c[:, :], xin[:, C_XT + ch * 1024 : C_XT + (ch + 1) * 1024]
                )
                xts.append(xt_c)
            xn_sb = cpool.tile([P, NT * D], bf16)
            nc.gpsimd.dma_start(xn_sb[:, :], xin[:, C_XN:C_END])
            WT_sb = prm[:, C_WT : C_WT + D]
            av_sb = prm[:, C_AV : C_AV + 2]

            # persistent sbuf state
            wxp = cpool.tile([P, MT * (D + 1)], bf16)   # [Wx | s1->ones]
            wxp_r = wxp.rearrange("p (m c) -> p m c", c=D + 1)
            r_b = cpool.tile([P, HALF], bf16)
            E1 = cpool.tile([P, MT], f32)
            F1 = cpool.tile([P, MT], f32)
            nE1 = cpool.tile([P, MT], f32)
            E1b = cpool.tile([P, MT], bf16)
            e1row_sb = cpool.tile([1, D + 1], bf16)
            ones1p = cpool.tile([1, P], bf16)

            with tc.tile_pool(name="ppsum", bufs=2, space="PSUM") as ppool:
                # ---- wproj = [W | w1] bf16; w2 broadcast tile ----
                wproj = cpool.tile([P, D + 1], bf16)
                nc.scalar.activation(wproj[:, :D], prm[:, C_W : C_W + D], Act.Copy)
                w_ps = ppool.tile([P, 2], f32, tag="w", name="w_ps", bufs=1)
                nc.tensor.matmul(w_ps[:, :], WT_sb, av_sb, start=True, stop=True)
                nc.scalar.activation(
                    wproj[:, D : D + 1], w_ps[:, 0:1], Act.Copy, scale=SCALE_INV
                )
                w2s = cpool.tile([P, 1], f32)
                nc.scalar.activation(
                    w2s[:, :], w_ps[:, 1:2], Act.Copy, scale=-0.99 * SCALE_INV
                )
                w2b = cpool.tile([P, P], bf16)
                nc.vector.tensor_scalar(
                    w2b[:, :], WT_sb, 0.0, w2s[:, :], Alu.mult, Alu.add
                )
                nc.vector.tensor_scalar(
                    ones1p[0:1, :], WT_sb[0:1, :], 0.0, 1.0, Alu.mult, Alu.add
                )

                # ---- r_b[p, n] = exp(-0.99 * s2[n]) replicated over p ----
                for q in range(4):
                    rb_ps = ppool.tile([P, 512], f32, tag="rb", name="rb_ps")
                    nc.tensor.matmul(
                        rb_ps[:, :],
                        w2b[:, :],
                        xts[q // 2][:, (q % 2) * 512 : (q % 2) * 512 + 512],
                        start=True,
                        stop=True,
                    )
                    nc.scalar.activation(
                        r_b[:, q * 512 : (q + 1) * 512], rb_ps[:, :], Act.Exp
                    )

                # ---- projection: p_ps = [Wx | s1]; cast into wxp ----
                for mt in range(MT):
                    p_ps = ppool.tile(
                        [P, D + 1], f32, tag="pp", name="p_ps", bufs=4
                    )
                    nc.tensor.matmul(
                        p_ps[:, :],
                        xts[mt // 8][:, (mt % 8) * P : (mt % 8) * P + P],
                        wproj[:, :],
                        start=True,
                        stop=True,
                    )
                    if mt % 2 == 0:
                        nc.vector.tensor_copy(
                            wxp_r[:, mt : mt + 1, :], p_ps[:, :]
                        )
                    else:
                        nc.scalar.activation(
                            wxp_r[:, mt : mt + 1, :], p_ps[:, :], Act.Copy
                        )

                # ---- batched exps from staged s1 (strided col view) ----
                s1v = wxp_r[:, :, D : D + 1]
                E1v = E1.rearrange("p (m o) -> p m o", o=1)
                F1v = F1.rearrange("p (m o) -> p m o", o=1)
                nc.scalar.activation(E1v, s1v, Act.Exp)
                nc.scalar.activation(F1v, s1v, Act.Exp, scale=0.01)
                # overwrite the s1 columns with 1.0 (denominator column)
                nc.vector.tensor_scalar(
                    s1v, E1v, 0.0, 1.0, Alu.mult, Alu.add
                )
                nc.vector.tensor_scalar(
                    nE1[:, :], E1[:, :], -1.0, None, Alu.mult
                )
                nc.scalar.activation(E1b[:, :], E1[:, :], Act.Copy)

                # ---- rank-1 E1 correction row for ACT-form tiles ----
                e1row_ps = ppool.tile([1, D + 1], f32, tag="er", name="e1row", bufs=1)
                for i, mt in enumerate(ACT_MTS):
                    nc.tensor.matmul(
                        e1row_ps[:, :],
                        E1b[:, mt : mt + 1],
                        wxp_r[:, mt : mt + 1, :],
                        start=(i == 0),
                        stop=(i == len(ACT_MTS) - 1),
                    )
                nc.scalar.activation(e1row_sb[:, :], e1row_ps[:, :], Act.Copy)

            # ---- main: score tiles + aggregation over q-pair halves ----
            with tc.tile_pool(name="mpsum", bufs=2, space="PSUM") as mpool:
                def accv(acc, j):
                    base = (j % 2) * (D + 1)
                    return acc[j // 2][:, base : base + D + 1]

                def finalize(qp, acc, o_h):
                    for j in range(8):
                        nt = qp * 8 + j
                        a = accv(acc, j)
                        rz = fpool.tile([P, 1], f32, tag="rz", name="rz")
                        nc.vector.reciprocal(rz[:, :], a[:, D : D + 1])
                        lr = fpool.tile([P, D], bf16, tag="lr", name="lr")
                        nc.scalar.activation(
                            lr[:, :],
                            a[:, :D],
                            Act.Prelu,
                            scale=rz[:, :],
                            alpha=0.01,
                        )
                        nc.gpsimd.tensor_tensor(
                            o_h[:, j * D : (j + 1) * D],
                            xn_sb[:, nt * D : (nt + 1) * D],
                            lr[:, :],
                            Alu.subtract,
                        )
                    out_view = out[qp * 1024 : (qp + 1) * 1024, :].rearrange(
                        "(j p) d -> p j d", p=P
                    )
                    nc.sync.dma_start(
                        out_view, o_h.rearrange("p (j d) -> p j d", j=8)
                    )

                pending = None
                for qp in range(2):
                    acc = [
                        mpool.tile(
                            [P, 2 * (D + 1)], f32, tag=f"acc{jj}", name=f"acc{jj}"
                        )
                        for jj in range(4)
                    ]
                    o_h = opool.tile([P, 8 * D], bf16, name="o_h")
                    for j in range(8):
                        nc.tensor.matmul(
                            accv(acc, j),
                            ones1p[0:1, :],
                            e1row_sb[0:1, :],
                            start=True,
                            stop=False,
                        )
                    for mt in range(MT):
                        rbs = r_b[:, qp * 1024 : (qp + 1) * 1024]
                        et = epool.tile([P, 1024], bf16, name="et")
                        eng = MT_ENG[mt]
                        if eng == "v":
                            nc.vector.tensor_scalar(
                                et[:, :], rbs,
                                F1[:, mt : mt + 1], E1[:, mt : mt + 1],
                                Alu.mult, Alu.max,
                            )
                        elif eng == "a":
                            nc.scalar.activation(
                                et[:, :], rbs, Act.Relu,
                                scale=F1[:, mt : mt + 1],
                                bias=nE1[:, mt : mt + 1],
                            )
                        else:
                            nc.gpsimd.tensor_scalar(
                                et[:, :], rbs,
                                F1[:, mt : mt + 1], E1[:, mt : mt + 1],
                                Alu.mult, Alu.max,
                            )
                        if mt == 4 and pending is not None:
                            finalize(*pending)
                            pending = None
                        for j in range(8):
                            nc.tensor.matmul(
                                accv(acc, j),
                                et[:, j * P : (j + 1) * P],
                                wxp_r[:, mt : mt + 1, :],
                                start=False,
                                stop=(mt == MT - 1),
                            )
                    pending = (qp, acc, o_h)
                finalize(*pending)

    nc.compile()
    return nc


def _prep_inputs(x, W, a1, a2):
    """Per-core packed bf16 input. Core c: t = c//2, n-half h = c%2.

    xT is host-rotated so the core's own 2048 dest columns come first
    (a rotation does not change a sum over all source nodes).
    """
    import ml_dtypes

    bf = ml_dtypes.bfloat16
    x = np.asarray(x, dtype=np.float32)
    W = np.ascontiguousarray(np.asarray(W, dtype=np.float32)).astype(bf)
    WT = np.ascontiguousarray(np.asarray(W, np.float32).T).astype(bf)
    av = np.ascontiguousarray(
        np.stack([np.asarray(a1, np.float32), np.asarray(a2, np.float32)], axis=1)
    ).astype(bf)
    xb = x.astype(bf)
    in_maps = []
    for c in range(8):
        t, h = c // 2, c % 2
        xt = xb[:, t, :].T  # [D, N]
        if h == 1:
            xt = np.concatenate([xt[:, HALF:], xt[:, :HALF]], axis=1)
        xn = xb[h * HALF : (h + 1) * HALF, t, :]  # [2048, 128]
        xn_packed = xn.reshape(NT, P, D).transpose(1, 0, 2).reshape(P, NT * D)
        xin = np.concatenate([W, WT, av, xt, xn_packed], axis=1)
        in_maps.append({"xin": np.ascontiguousarray(xin)})
    return in_maps


def _run(x, W, a1, a2, trace=False):
    from concourse.bass_utils import run_bass_kernel_spmd

    key = "nc"
    if key not in _CACHE:
        _CACHE[key] = _build()
    nc = _CACHE[key]
    in_maps = _prep_inputs(x, W, a1, a2)
    res = run_bass_kernel_spmd(nc, in_maps, list(range(8)), trace=trace)
    out_full = np.empty((N, T, D), dtype=np.float32)
    for c in range(8):
        t, h = c // 2, c % 2
        out_full[h * HALF : (h + 1) * HALF, t, :] = np.asarray(
            res.results[c]["out"]
        ).astype(np.float32)
    return out_full, res


def kernel(x, W, a1, a2):
    out, _ = _run(x, W, a1, a2, trace=False)
    return out


# revision 9
# speedup vs baseline: 2.4850x; 2.4850x over previous
"""GAT message-passing kernel for 8 trn2 NeuronCores.

Math (reference):
    Wx = x @ W;  s1 = Wx@a1/sqrt(2D);  s2 = Wx@a2/sqrt(2D)   (per t)
    weight = softmax_m(lrelu(s1[m] + s2[n]));  agg = lrelu(weight @ Wx)
    out = x - agg

Key identities:
  * exp(lrelu(s1+s2)) rescaled by exp(-s2) gives per-(m,n) weight
        et[m,n] = max(E1[m], F1[m] * r[n])
    with E1=exp(s1), F1=exp(0.01 s1), r=exp(-0.99 s2) - O(N) exps only.
  * et = E1[m] + relu(F1[m]*r[n] - E1[m]), so the score tile can be
    produced EITHER as one DVE/Pool tensor_scalar (mult,max) or as one
    ACT Relu with per-partition scale/bias; the E1 part for ACT-form
    tiles is a rank-1 correction folded into the PSUM accumulation via
    a 1-partition broadcast matmul.
  * softmax denominator = ones column appended to Wx (moving operand).
  * out = x - prelu(agg, 0.01).

Sharding: 8 cores = 4 t-slices x 2 N-halves; each core aggregates over
all 4096 source nodes for its own (t, 2048 dest nodes). Whole data
path in bf16 (inputs packed bf16 host-side, output bf16 -> f32).
"""

import sys

if "/opt/trn_rl_repo" not in sys.path:
    sys.path.insert(0, "/opt/trn_rl_repo")

import numpy as np

N, T, D = 4096, 4, 128
P = 128
HALF = N // 2            # 2048 dest nodes per core
MT = N // P              # 32 m tiles
NT = HALF // P           # 16 own n tiles
SCALE_INV = 1.0 / 16.0   # 1/sqrt(2*128)

# packed bf16 input column offsets: [W | WT | av | xT | xn]
C_W = 0
C_WT = D                 # 128
C_AV = 2 * D             # 256
C_XT = 2 * D + 2         # 258
C_XN = C_XT + N          # 4354
C_END = C_XN + HALF      # 6402

# per-mt engine for the score tile: v=DVE (max form), a=ACT (relu form).
# ACT-form tiles need the rank-1 E1 correction row.
MT_ENG = list("vvavvava" * 4)
ACT_MTS = [i for i, e in enumerate(MT_ENG) if e == "a"]

_CACHE = {}


def _build():
    import concourse.mybir as mybir
    from concourse import bacc
    from concourse.tile import TileContext

    f32 = mybir.dt.float32
    bf16 = mybir.dt.bfloat16
    Alu = mybir.AluOpType
    Act = mybir.ActivationFunctionType

    nc = bacc.Bacc()
    xin = nc.declare_dram_parameter("xin", [P, C_END], bf16, isOutput=False)
    out = nc.declare_dram_parameter("out", [HALF, D], bf16, isOutput=True)

    with TileContext(nc) as tc:
        with (
            tc.tile_pool(name="const", bufs=1) as cpool,
            tc.tile_pool(name="epool", bufs=6) as epool,
            tc.tile_pool(name="fpool", bufs=4) as fpool,
            tc.tile_pool(name="opool", bufs=2) as opool,
        ):
            # ---- input DMAs ----
            px0 = cpool.tile([P, C_XT + 1024], bf16)
            nc.sync.dma_start(px0[:, :], xin[:, 0 : C_XT + 1024])
            prm = px0[:, 0:C_XT]
            xts = [px0[:, C_XT : C_XT + 1024]]
            for ch in range(1, 4):
                xt_c = cpool.tile([P, 1024], bf16, name=f"xt{ch}", tag=f"xt{ch}")
                nc.sync.dma_start(
                    xt_c[:, :], xin[:, C_XT + ch * 1024 : C_XT + (ch + 1) * 1024]
                )
                xts.append(xt_c)
            xn_sb = cpool.tile([P, NT * D], bf16)
            nc.gpsimd.dma_start(xn_sb[:, :], xin[:, C_XN:C_END])
            WT_sb = prm[:, C_WT : C_WT + D]
            av_sb = prm[:, C_AV : C_AV + 2]

            # persistent sbuf state
            wxp = cpool.tile([P, MT * (D + 1)], bf16)   # [Wx | s1->ones]
            wxp_r = wxp.rearrange("p (m c) -> p m c", c=D + 1)
            r_b = cpool.tile([P, HALF], bf16)
            E1 = cpool.tile([P, MT], f32)
            F1 = cpool.tile([P, MT], f32)
            nE1 = cpool.tile([P, MT], f32)
            E1b = cpool.tile([P, MT], bf16)
            e1row_sb = cpool.tile([1, D + 1], bf16)
            ones1p = cpool.tile([1, P], bf16)

            with tc.tile_pool(name="ppsum", bufs=2, space="PSUM") as ppool:
                # ---- wproj = [W | w1] bf16; w2 broadcast tile ----
                wproj = cpool.tile([P, D + 1], bf16)
                nc.scalar.activation(wproj[:, :D], prm[:, C_W : C_W + D], Act.Copy)
                w_ps = ppool.tile([P, 2], f32, tag="w", name="w_ps", bufs=1)
                nc.tensor.matmul(w_ps[:, :], WT_sb, av_sb, start=True, stop=True)
                nc.scalar.activation(
                    wproj[:, D : D + 1], w_ps[:, 0:1], Act.Copy, scale=SCALE_INV
                )
                w2s = cpool.tile([P, 1], f32)
                nc.scalar.activation(
                    w2s[:, :], w_ps[:, 1:2], Act.Copy, scale=-0.99 * SCALE_INV
                )
                w2b = cpool.tile([P, P], bf16)
                nc.vector.tensor_scalar(
                    w2b[:, :], WT_sb, 0.0, w2s[:, :], Alu.mult, Alu.add
                )
                nc.vector.tensor_scalar(
                    ones1p[0:1, :], WT_sb[0:1, :], 0.0, 1.0, Alu.mult, Alu.add
                )

                # ---- r_b[p, n] = exp(-0.99 * s2[n]) replicated over p ----
                for q in range(4):
                    rb_ps = ppool.tile([P, 512], f32, tag="rb", name="rb_ps")
                    nc.tensor.matmul(
                        rb_ps[:, :],
                        w2b[:, :],
                        xts[q // 2][:, (q % 2) * 512 : (q % 2) * 512 + 512],
                        start=True,
                        stop=True,
                    )
                    nc.scalar.activation(
                        r_b[:, q * 512 : (q + 1) * 512], rb_ps[:, :], Act.Exp
                    )

                # ---- projection: p_ps = [Wx | s1]; cast into wxp ----
                for mt in range(MT):
                    p_ps = ppool.tile(
                        [P, D + 1], f32, tag="pp", name="p_ps", bufs=4
                    )
                    nc.tensor.matmul(
                        p_ps[:, :],
                        xts[mt // 8][:, (mt % 8) * P : (mt % 8) * P + P],
                        wproj[:, :],
                        start=True,
                        stop=True,
                    )
                    if mt % 2 == 0:
                        nc.vector.tensor_copy(
                            wxp_r[:, mt : mt + 1, :], p_ps[:, :]
                        )
                    else:
                        nc.scalar.activation(
                            wxp_r[:, mt : mt + 1, :], p_ps[:, :], Act.Copy
                        )

                # ---- batched exps from staged s1 (strided col view) ----
                s1v = wxp_r[:, :, D : D + 1]
                E1v = E1.rearrange("p (m o) -> p m o", o=1)
                F1v = F1.rearrange("p (m o) -> p m o", o=1)
                nc.scalar.activation(E1v, s1v, Act.Exp)
                nc.scalar.activation(F1v, s1v, Act.Exp, scale=0.01)
                # overwrite the s1 columns with 1.0 (denominator column)
                nc.vector.tensor_scalar(
                    s1v, E1v, 0.0, 1.0, Alu.mult, Alu.add
                )
                nc.vector.tensor_scalar(
                    nE1[:, :], E1[:, :], -1.0, None, Alu.mult
                )
                nc.scalar.activation(E1b[:, :], E1[:, :], Act.Copy)

                # ---- rank-1 E1 correction row for ACT-form tiles ----
                e1row_ps = ppool.tile([1, D + 1], f32, tag="er", name="e1row", bufs=1)
                for i, mt in enumerate(ACT_MTS):
                    nc.tensor.matmul(
                        e1row_ps[:, :],
                        E1b[:, mt : mt + 1],
                        wxp_r[:, mt : mt + 1, :],
                        start=(i == 0),
                        stop=(i == len(ACT_MTS) - 1),
                    )
                nc.scalar.activation(e1row_sb[:, :], e1row_ps[:, :], Act.Copy)

            # ---- main: score tiles + aggregation per 512-dest chunk ----
            with tc.tile_pool(name="mpsum", bufs=2, space="PSUM") as mpool:
                def accv(acc, j):
                    base = (j % 2) * (D + 1)
                    return acc[j // 2][:, base : base + D + 1]

                def finalize(q, acc, o_q):
                    for j in range(4):
                        nt = q * 4 + j
                        a = accv(acc, j)
                        rz = fpool.tile([P, 1], f32, tag="rz", name="rz")
                        nc.vector.reciprocal(rz[:, :], a[:, D : D + 1])
                        lr = fpool.tile([P, D], bf16, tag="lr", name="lr")
                        nc.scalar.activation(
                            lr[:, :],
                            a[:, :D],
                            Act.Prelu,
                            scale=rz[:, :],
                            alpha=0.01,
                        )
                        nc.gpsimd.tensor_tensor(
                            o_q[:, j * D : (j + 1) * D],
                            xn_sb[:, nt * D : (nt + 1) * D],
                            lr[:, :],
                            Alu.subtract,
                        )
                    out_view = out[q * 512 : (q + 1) * 512, :].rearrange(
                        "(j p) d -> p j d", p=P
                    )
                    nc.sync.dma_start(
                        out_view, o_q.rearrange("p (j d) -> p j d", j=4)
                    )

                pending = None
                for q in range(4):
                    acc = [
                        mpool.tile(
                            [P, 2 * (D + 1)], f32, tag=f"acc{jj}", name=f"acc{jj}"
                        )
                        for jj in range(2)
                    ]
                    o_q = opool.tile([P, 4 * D], bf16, name="o_q")
                    for j in range(4):
                        nc.tensor.matmul(
                            accv(acc, j),
                            ones1p[0:1, :],
                            e1row_sb[0:1, :],
                            start=True,
                            stop=False,
                        )
                    for mt in range(MT):
                        rbs = r_b[:, q * 512 : (q + 1) * 512]
                        et = epool.tile([P, 512], bf16, name="et")
                        eng = MT_ENG[mt]
                        if eng == "v":
                            nc.vector.tensor_scalar(
                                et[:, :], rbs,
                                F1[:, mt : mt + 1], E1[:, mt : mt + 1],
                                Alu.mult, Alu.max,
                            )
                        else:
                            nc.scalar.activation(
                                et[:, :], rbs, Act.Relu,
                                scale=F1[:, mt : mt + 1],
                                bias=nE1[:, mt : mt + 1],
                            )
                        if mt == 4 and pending is not None:
                            finalize(*pending)
                            pending = None
                        for j in range(4):
                            nc.tensor.matmul(
                                accv(acc, j),
                                et[:, j * P : (j + 1) * P],
                                wxp_r[:, mt : mt + 1, :],
                                start=False,
                                stop=(mt == MT - 1),
                            )
                    pending = (q, acc, o_q)
                finalize(*pending)

    nc.compile()
    return nc


def _prep_inputs(x, W, a1, a2):
    """Per-core packed bf16 input. Core c: t = c//2, n-half h = c%2.

    xT is host-rotated so the core's own 2048 dest columns come first
    (a rotation does not change a sum over all source nodes).
    """
    import ml_dtypes

    bf = ml_dtypes.bfloat16
    x = np.asarray(x, dtype=np.float32)
    W = np.ascontiguousarray(np.asarray(W, dtype=np.float32)).astype(bf)
    WT = np.ascontiguousarray(np.asarray(W, np.float32).T).astype(bf)
    av = np.ascontiguousarray(
        np.stack([np.asarray(a1, np.float32), np.asarray(a2, np.float32)], axis=1)
    ).astype(bf)
    xb = x.astype(bf)
    in_maps = []
    for c in range(8):
        t, h = c // 2, c % 2
        xt = xb[:, t, :].T  # [D, N]
        if h == 1:
            xt = np.concatenate([xt[:, HALF:], xt[:, :HALF]], axis=1)
        xn = xb[h * HALF : (h + 1) * HALF, t, :]  # [2048, 128]
        xn_packed = xn.reshape(NT, P, D).transpose(1, 0, 2).reshape(P, NT * D)
        xin = np.concatenate([W, WT, av, xt, xn_packed], axis=1)
        in_maps.append({"xin": np.ascontiguousarray(xin)})
    return in_maps


def _run(x, W, a1, a2, trace=False):
    from concourse.bass_utils import run_bass_kernel_spmd

    key = "nc"
    if key not in _CACHE:
        _CACHE[key] = _build()
    nc = _CACHE[key]
    in_maps = _prep_inputs(x, W, a1, a2)
    res = run_bass_kernel_spmd(nc, in_maps, list(range(8)), trace=trace)
    out_full = np.empty((N, T, D), dtype=np.float32)
    for c in range(8):
        t, h = c // 2, c % 2
        out_full[h * HALF : (h + 1) * HALF, t, :] = np.asarray(
            res.results[c]["out"]
        ).astype(np.float32)
    return out_full, res


def kernel(x, W, a1, a2):
    out, _ = _run(x, W, a1, a2, trace=False)
    return out
